# revision 1
# baseline (speedup 1.0000x reference)
"""Multi-head causal attention on 8 Trainium2 NeuronCores.

Problem: resid_pre [4, 2048, 1024], 16 heads x d_head 64, causal softmax,
output [4, 2048, 1024] f32.

Sharding: data-parallel over the 4 batches x tensor-parallel over 2 head
groups (8 heads each) -> 8 cores. Each core computes the attention output
contribution of its 8 heads for its batch; the host sums the two head-group
partials per batch (the "all-reduce") and adds the output bias.

Per-core kernel (matmul inputs bf16, all accumulation fp32 in PSUM;
measured ~3.5e-3 max rel err vs the fp32 reference):

  prelude, pipelined by 512-column blocks of X^T (causality means attention
  superblock sb only needs Q/K columns <= (sb+1)*512):
    V = X @ W_v for all 8 heads in natural [seq, d] layout with a ones
    column appended per head, and Q^T/K^T for head pair 0, pair-stacked on
    partitions (head 2p in partitions 0-63, 2p+1 in 64-127).

  per head pair p (heads 2p, 2p+1), per 512-wide query superblock, per
  128-wide key tile:
    S^T = K^T.T @ Q^T (keys on partitions, one matmul per head via
    partition row groups), restricted to the un-masked column suffix;
    causal triangle added to the diagonal block in-PSUM via an
    identity-stationary matmul; exp on ScalarE (no max subtraction needed,
    scores are O(1)); z~^T[65, 512] += V_chunk.T @ P~^T accumulated in
    PSUM, whose row 64 (from the ones column) is the softmax denominator;
    normalize with reciprocal_approx_fast + gpsimd partition broadcast.
    Pair p+1's Q/K projection matmuls are interleaved into this ACT-paced
    stream so the PE never starves; for the last pair the output
    projection tiles of already-final superblocks are interleaved instead.

  output projection: out[q, m] = sum_p z^T_p.T @ W_o_p, PSUM -> SBUF ->
  DRAM.

b_Q/b_K are applied on-device (per-partition bias during the PSUM->SBUF
copy); b_V's exact contribution sum_h W_O[h].T @ b_V[h] (softmax rows sum
to 1) and b_O are added on the host.
"""
import ml_dtypes
import numpy as np

import concourse.bass as bass
import concourse.mybir as mybir
import concourse.tile as tile
from concourse import bacc
from concourse import bass_utils

F32 = mybir.dt.float32
F32R = mybir.dt.float32r
EXPF = mybir.ActivationFunctionType.Exp

S = 2048          # sequence length
DM = 1024         # d_model
DH = 64           # d_head
NHC = 8           # heads per core
PAIRS = 4         # head pairs per core
MC = 8            # d_model chunks of 128
NSB = 4           # query superblocks of 512
SBW = 512         # superblock width
NKT = 16          # key tiles of 128
NST = 16          # seq tiles of 128
MASK_NEG = -1e9
SCALE = 0.125     # 1/sqrt(d_head)

_NC_CACHE = {}
LAST_RESULTS = None


def _build_nc():
    nc = bacc.Bacc("TRN2", target_bir_lowering=False, debug=False)
    BF16 = mybir.dt.bfloat16
    xt_d = nc.dram_tensor("xt", [DM, S], BF16, kind="ExternalInput")
    wq_d = nc.dram_tensor("wq", [PAIRS, MC, 128, 128], BF16, kind="ExternalInput")
    wk_d = nc.dram_tensor("wk", [PAIRS, MC, 128, 128], BF16, kind="ExternalInput")
    wv_d = nc.dram_tensor("wv", [MC, 128, NHC * DH], BF16, kind="ExternalInput")
    wo_d = nc.dram_tensor("wo", [PAIRS, 128, DM], BF16, kind="ExternalInput")
    bq_d = nc.dram_tensor("bq", [PAIRS, 128, 1], F32, kind="ExternalInput")
    bk_d = nc.dram_tensor("bk", [PAIRS, 128, 1], F32, kind="ExternalInput")
    msk_d = nc.dram_tensor("mask", [128, 128], BF16, kind="ExternalInput")
    id_d = nc.dram_tensor("ident", [128, 128], BF16, kind="ExternalInput")
    out_d = nc.dram_tensor("out", [S, DM], F32, kind="ExternalOutput")

    with tile.TileContext(nc) as tc:
      with (
          tc.tile_pool(name="hold", bufs=1) as hold,
          tc.tile_pool(name="ph2", bufs=1) as ph2,
          tc.tile_pool(name="patn", bufs=1, space="PSUM") as patn,
      ):
        v_t = [hold.tile([128, NHC, DH + 1], BF16, tag=f"v{st}", name=f"v{st}") for st in range(NST)]
        z_t = [hold.tile([128, S], BF16, tag=f"z{p}", name=f"z{p}") for p in range(PAIRS)]
        msk_t = hold.tile([128, 128], BF16, tag="mtri")
        id_t = hold.tile([128, 128], BF16, tag="ident")
        bq_t = [hold.tile([128, 1], F32, tag=f"bq{p}", name=f"bq{p}") for p in range(PAIRS)]
        bk_t = [hold.tile([128, 1], F32, tag=f"bk{p}", name=f"bk{p}") for p in range(PAIRS)]
        ones_c = hold.tile([128, 1], F32, tag="ones")
        qts = {}

        nc.vector.memset(ones_c[:], 1.0)
        # small constants go through the (otherwise idle) gpsimd DMA queue so
        # their triggers don't delay the xt/wv bulk loads
        nc.gpsimd.dma_start(msk_t[:], msk_d.ap())
        nc.gpsimd.dma_start(id_t[:], id_d.ap())
        for p in range(PAIRS):
            nc.gpsimd.dma_start(bq_t[p][:], bq_d.ap()[p])
            nc.gpsimd.dma_start(bk_t[p][:], bk_d.ap()[p])

        def attn_j(p, sb, j, z0, z1):
            qt, kt = qts[p]
            qtb = qt[sb]
            ktb = kt[j // 4]
            nkt = 4 * (sb + 1)
            # columns q < j*128 of this key tile are fully masked;
            # restrict S/exp/PV to the valid suffix.
            j_rel = j - 4 * sb
            off = max(j_rel, 0) * 128
            sp = patn.tile([128, 1024], F32, tag="sp", bufs=2, name="sp")
            ks = ((j % 4) * 128, (j % 4 + 1) * 128)
            diag = j_rel >= 0
            nc.tensor.matmul(
                sp[:, off:512],
                ktb[0:64, ks[0]:ks[1]],
                qtb[0:64, off:SBW],
                start=True, stop=not diag,
                tile_position=(0, 0),
                skip_group_check=True,
            )
            nc.tensor.matmul(
                sp[:, 512 + off:1024],
                ktb[64:128, ks[0]:ks[1]],
                qtb[64:128, off:SBW],
                start=True, stop=not diag,
                tile_position=(64, 0),
                skip_group_check=True,
            )
            if diag:
                # add the causal triangle to the diagonal block in-PSUM:
                # out += I.T @ mask  (PE accumulate, no DVE on critical path)
                for u in (0, 1):
                    lo = u * 512 + off
                    nc.tensor.matmul(
                        sp[:, lo:lo + 128],
                        id_t[:],
                        msk_t[:],
                        start=False, stop=True,
                        skip_group_check=True,
                    )
            pt = ph2.tile([128, 1024], BF16, tag="pt", bufs=6, name="pt")
            sp3 = sp[:].rearrange("p (u q) -> p u q", u=2)
            pt3 = pt[:].rearrange("p (u q) -> p u q", u=2)
            nc.scalar.activation(
                pt3[:, :, off:512], sp3[:, :, off:512], EXPF, scale=SCALE
            )
            nc.tensor.matmul(
                z0[:, off:512],
                v_t[j][:, 2 * p, :],
                pt[:, off:512],
                start=(j == 0), stop=(j == nkt - 1),
            )
            nc.tensor.matmul(
                z1[:, off:512],
                v_t[j][:, 2 * p + 1, :],
                pt[:, 512 + off:1024],
                start=(j == 0), stop=(j == nkt - 1),
            )

        def attn_norm(p, sb, z0, z1):
            # normalize by the softmax denominator (row DH of z psum).
            # First copy z psum to SBUF so the bank frees immediately (the
            # next superblock's PV only waits for this copy, not the whole
            # reciprocal/broadcast/multiply chain).
            qs = (sb * SBW, (sb + 1) * SBW)
            d0row = ph2.tile([1, 512], F32, tag="d0row", bufs=2, name="d0row")
            d1row = ph2.tile([1, 512], F32, tag="d1row", bufs=2, name="d1row")
            nc.vector.tensor_copy(d0row[:], z0[DH:DH + 1, :])
            nc.vector.tensor_copy(d1row[:], z1[DH:DH + 1, :])
            nc.vector.reciprocal_approx_fast(d0row[:], d0row[:])
            nc.vector.reciprocal_approx_fast(d1row[:], d1row[:])
            r0 = ph2.tile([64, 512], F32, tag="r0", bufs=2, name="r0")
            r1 = ph2.tile([64, 512], F32, tag="r1", bufs=2, name="r1")
            nc.gpsimd.partition_broadcast(r0[:], d0row[:], channels=64)
            nc.gpsimd.partition_broadcast(r1[:], d1row[:], channels=64)
            nc.vector.tensor_mul(z_t[p][0:64, qs[0]:qs[1]], z0[0:64, :], r0[:])
            t1 = ph2.tile([64, 512], BF16, tag="t1", bufs=2, name="t1")
            nc.vector.tensor_mul(t1[:], z1[0:64, :], r1[:])
            nc.sync.dma_start(z_t[p][64:128, qs[0]:qs[1]], t1[:])

        with (
            tc.tile_pool(name="ph1", bufs=1) as ph1,
            tc.tile_pool(name="pqk", bufs=1, space="PSUM") as pqk,
        ):
            # xt in per-512-column-block tiles: attention(0, sb) needs only
            # Q/K columns <= (sb+1)*512 (causal), so the whole front of the
            # kernel pipelines by column block.
            xt_t = [[ph1.tile([128, SBW], BF16, tag=f"xt{m}_{cb}", name=f"xt{m}_{cb}")
                     for cb in range(NSB)] for m in range(MC)]
            wv_t = [ph1.tile([128, NHC * DH], BF16, tag=f"wv{m}", name=f"wv{m}") for m in range(MC)]

            def qk_gen(p, sb_outer=False):
                """QK projection for pair p (bf16, pair-stacked partitions),
                yielded one matmul at a time for interleaving. With
                sb_outer=True the superblock loop is outermost so early
                superblocks finish as soon as their xt column block lands."""
                qt = [hold.tile([128, SBW], BF16, tag=f"qt{i}", bufs=2, name=f"qt{i}")
                      for i in range(NSB)]
                kt = [hold.tile([128, SBW], BF16, tag=f"kt{i}", bufs=2, name=f"kt{i}")
                      for i in range(NSB)]
                qts[p] = (qt, kt)
                wqk = []
                for (w_d, b_t, dst) in ((wq_d, bq_t, qt), (wk_d, bk_t, kt)):
                    wts = []
                    for m in range(MC):
                        w = ph1.tile([128, 128], BF16, tag="w", bufs=16, name="w")
                        nc.sync.dma_start(w[:], w_d.ap()[p, m])
                        wts.append(w)
                    wqk.append((wts, b_t, dst))
                order = (
                    [(sb, wb) for sb in range(NSB) for wb in wqk]
                    if sb_outer else
                    [(sb, wb) for wb in wqk for sb in range(NSB)]
                )
                for sb, (wts, b_t, dst) in order:
                    ps = pqk.tile([128, 512], F32, tag="acc", bufs=2, name="acc")
                    for m in range(MC):
                        nc.tensor.matmul(
                            ps[:],
                            wts[m][:],
                            xt_t[m][sb][:],
                            start=(m == 0),
                            stop=(m == MC - 1),
                        )
                        yield
                    nc.vector.tensor_scalar_add(dst[sb][:], ps[:], b_t[p][:])
                    yield

            # column-block pipelined prelude: per block, land xt columns,
            # then V-projection for its 4 seq tiles and pair 0's QK for it.
            g0 = qk_gen(0, sb_outer=True)
            for cb in range(NSB):
                for m in range(MC):
                    # alternate trigger queues: each dma_start costs ~0.6us of
                    # issuing-engine queue time, which otherwise serializes
                    eng = nc.scalar if m % 2 == 0 else nc.sync
                    eng.dma_start(
                        xt_t[m][cb][:],
                        xt_d.ap()[m * 128:(m + 1) * 128, cb * SBW:(cb + 1) * SBW],
                    )
                    if cb == 0:
                        eng2 = nc.sync if m % 2 == 0 else nc.scalar
                        eng2.dma_start(wv_t[m][:], wv_d.ap()[m])
                for st in range(4 * cb, 4 * cb + 4):
                    ps = pqk.tile([128, 512], F32, tag="acc", bufs=2, name="acc")
                    for m in range(MC):
                        nc.tensor.matmul(
                            ps[:],
                            xt_t[m][cb][:, (st % 4) * 128:(st % 4 + 1) * 128],
                            wv_t[m][:],
                            start=(m == 0),
                            stop=(m == MC - 1),
                        )
                    nc.vector.tensor_copy(
                        v_t[st][:, :, 0:DH],
                        ps[:].rearrange("p (h d) -> p h d", h=NHC),
                    )
                    nc.vector.tensor_copy(
                        v_t[st][:, :, DH],
                        ones_c[:].to_broadcast((128, NHC)),
                    )
                for _ in range(18):  # one QK column-block (2 proj x (8 mm + copy))
                    try:
                        next(g0)
                    except StopIteration:
                        break
            for _ in g0:
                pass

            # attention for pairs 0-2, with pair p+1's projection matmuls
            # interleaved into the ACT-paced attention stream
            for p in range(3):
                g = qk_gen(p + 1)
                done = False
                emitted = 0
                step = 0
                for sb in range(NSB):
                    nkt = 4 * (sb + 1)
                    z0 = patn.tile([DH + 1, 512], F32, tag="z0", bufs=1, name="z0")
                    z1 = patn.tile([DH + 1, 512], F32, tag="z1", bufs=1, name="z1")
                    for j in range(nkt):
                        attn_j(p, sb, j, z0, z1)
                        step += 1
                        want = 2 * step if step <= 32 else 64 + (step - 32)
                        while emitted < want and not done:
                            try:
                                next(g)
                                emitted += 1
                            except StopIteration:
                                done = True
                    attn_norm(p, sb, z0, z1)
                while not done:
                    try:
                        next(g)
                    except StopIteration:
                        done = True

        # ---------------- last pair + output projection ----------------
        with tc.tile_pool(name="ph3", bufs=1) as ph3:
            wo_t = [ph3.tile([128, DM], BF16, tag=f"wo{p}", name=f"wo{p}") for p in range(PAIRS)]
            for p in range(PAIRS):
                nc.sync.dma_start(wo_t[p][:], wo_d.ap()[p])

            def oproj(q, mb, tag="sp", cp=None):
                ps = patn.tile([128, 512], F32, tag=tag, bufs=2 if tag == "sp" else 1, name="ops")
                for p in range(PAIRS):
                    nc.tensor.matmul(
                        ps[:],
                        z_t[p][:, q * 128:(q + 1) * 128],
                        wo_t[p][:, mb * 512:(mb + 1) * 512],
                        start=(p == 0),
                        stop=(p == PAIRS - 1),
                    )
                ost = ph3.tile([128, 512], F32, tag="ost", bufs=4, name="ost")
                (cp or nc.vector.tensor_copy)(ost[:], ps[:])
                nc.sync.dma_start(
                    out_d.ap()[q * 128:(q + 1) * 128, mb * 512:(mb + 1) * 512],
                    ost[:],
                )

            # pair 3's attention, with output-projection tiles for already-
            # complete superblocks interleaved in (sb lags by one).
            otodo = [(q, mb) for q in range(NST) for mb in range(2)]
            odone = 0
            # alternate z psum between two tag sets (the second lives in the
            # banks freed by the projection pool) so superblock boundaries
            # don't stall on the previous normalize.
            for sb in range(NSB):
                nkt = 4 * (sb + 1)
                z0 = patn.tile([DH + 1, 512], F32, tag="z0", bufs=1, name="z0")
                z1 = patn.tile([DH + 1, 512], F32, tag="z1", bufs=1, name="z1")
                for j in range(nkt):
                    attn_j(3, sb, j, z0, z1)
                    # z for superblocks < sb is final for all pairs
                    ready = sb * 8
                    if odone < ready:
                        oproj(*otodo[odone])
                        odone += 1
                attn_norm(3, sb, z0, z1)
            # final groups (need the last superblock's z): emit p-major in
            # waves of 4 so the pair-0..2 matmuls run during the attention
            # tail and only the final per-group matmul waits on the last
            # normalize; copies alternate between the two idle engines.
            rest = otodo[odone:]
            for w0 in range(0, len(rest), 4):
                wave = rest[w0:w0 + 4]
                tags = ["sp", "sp", "z0", "z1"]
                pss = [
                    patn.tile([128, 512], F32, tag=tags[i],
                              bufs=2 if tags[i] == "sp" else 1, name="opsf")
                    for i in range(len(wave))
                ]
                for p in range(PAIRS):
                    for (q, mb), ps in zip(wave, pss):
                        nc.tensor.matmul(
                            ps[:],
                            z_t[p][:, q * 128:(q + 1) * 128],
                            wo_t[p][:, mb * 512:(mb + 1) * 512],
                            start=(p == 0),
                            stop=(p == PAIRS - 1),
                            skip_group_check=True,
                        )
                for i, ((q, mb), ps) in enumerate(zip(wave, pss)):
                    ost = ph3.tile([128, 512], F32, tag="ost", bufs=4, name="ost")
                    cp = nc.scalar.copy if i % 2 == 0 else nc.vector.tensor_copy
                    cp(ost[:], ps[:])
                    nc.sync.dma_start(
                        out_d.ap()[q * 128:(q + 1) * 128, mb * 512:(mb + 1) * 512],
                        ost[:],
                    )

    nc.compile()
    return nc


def _get_nc():
    if "nc" not in _NC_CACHE:
        _NC_CACHE["nc"] = _build_nc()
    return _NC_CACHE["nc"]


def _causal_masks():
    k = np.arange(128)[:, None]
    q = np.arange(128)[None, :]
    return np.where(q >= k, 0.0, MASK_NEG).astype(ml_dtypes.bfloat16)


def kernel(resid_pre, W_Q, W_K, W_V, W_O, b_Q, b_K, b_V, b_O):
    global LAST_RESULTS
    resid_pre = np.asarray(resid_pre, dtype=np.float32)
    W_Q = np.asarray(W_Q, dtype=np.float32)
    W_K = np.asarray(W_K, dtype=np.float32)
    W_V = np.asarray(W_V, dtype=np.float32)
    W_O = np.asarray(W_O, dtype=np.float32)
    b_Q = np.asarray(b_Q, dtype=np.float32)
    b_K = np.asarray(b_K, dtype=np.float32)
    b_V = np.asarray(b_V, dtype=np.float32)
    b_O = np.asarray(b_O, dtype=np.float32)

    B = resid_pre.shape[0]
    masks = _causal_masks()
    ident = np.eye(128, dtype=ml_dtypes.bfloat16)

    def pack_pairs(w):  # [8, 1024, 64] -> [4, 8, 128, 128]
        return np.ascontiguousarray(
            w.reshape(PAIRS, 2, DM, DH).transpose(0, 2, 1, 3).reshape(PAIRS, MC, 128, 128)
        )

    in_maps = []
    for c in range(8):
        b, g = divmod(c, 2)
        hs = slice(g * NHC, (g + 1) * NHC)
        in_maps.append({
            "xt": np.ascontiguousarray(resid_pre[b].T).astype(ml_dtypes.bfloat16),
            "wq": pack_pairs(W_Q[hs]).astype(ml_dtypes.bfloat16),
            "wk": pack_pairs(W_K[hs]).astype(ml_dtypes.bfloat16),
            "wv": np.ascontiguousarray(
                W_V[hs].transpose(1, 0, 2).reshape(DM, NHC * DH).reshape(MC, 128, NHC * DH)
            ).astype(ml_dtypes.bfloat16),
            "wo": np.ascontiguousarray(W_O[hs].reshape(PAIRS, 128, DM)).astype(ml_dtypes.bfloat16),
            "bq": np.ascontiguousarray(b_Q[hs].reshape(PAIRS, 128, 1)),
            "bk": np.ascontiguousarray(b_K[hs].reshape(PAIRS, 128, 1)),
            "mask": masks,
            "ident": ident,
        })

    nc = _get_nc()
    res = bass_utils.run_bass_kernel_spmd(nc, in_maps, core_ids=list(range(8)))
    LAST_RESULTS = res

    # b_V contributes exactly sum_h W_O[h].T @ b_V[h] (softmax rows sum to 1)
    const = np.einsum("hdm,hd->m", W_O, b_V).astype(np.float32) + b_O
    out = np.empty((B, S, DM), dtype=np.float32)
    for b in range(B):
        out[b] = res.results[2 * b]["out"] + res.results[2 * b + 1]["out"] + const
    return out



# revision 11
# speedup vs baseline: 1.1544x; 1.1544x over previous
"""Multi-head causal attention on 8 Trainium2 NeuronCores.

Problem: resid_pre [4, 2048, 1024], 16 heads x d_head 64, causal softmax,
output [4, 2048, 1024] f32.

Sharding: data-parallel over the 4 batches x tensor-parallel over 2 head
groups (8 heads each) -> 8 cores. Each core computes the attention output
contribution of its 8 heads for its batch; the host sums the two head-group
partials per batch (the "all-reduce") and adds the output bias.

Per-core kernel (matmul inputs bf16, all accumulation fp32 in PSUM;
measured ~3.5e-3 max rel err vs the fp32 reference):

  prelude, pipelined by 512-column blocks of X^T (causality means attention
  superblock sb only needs Q/K columns <= (sb+1)*512):
    V = X @ W_v for all 8 heads in natural [seq, d] layout with a ones
    column appended per head, and Q^T/K^T for head pair 0, pair-stacked on
    partitions (head 2p in partitions 0-63, 2p+1 in 64-127).

  per head pair p (heads 2p, 2p+1), per 512-wide query superblock, per
  128-wide key tile:
    S^T = K^T.T @ Q^T (keys on partitions, one matmul per head via
    partition row groups), restricted to the un-masked column suffix;
    causal triangle added to the diagonal block in-PSUM via an
    identity-stationary matmul; exp on ScalarE (no max subtraction needed,
    scores are O(1)); z~^T[65, 512] += V_chunk.T @ P~^T accumulated in
    PSUM, whose row 64 (from the ones column) is the softmax denominator;
    normalize with reciprocal_approx_fast + gpsimd partition broadcast.
    Pair p+1's Q/K projection matmuls are interleaved into this ACT-paced
    stream so the PE never starves; for the last pair the output
    projection tiles of already-final superblocks are interleaved instead.

  output projection: out[q, m] = sum_p z^T_p.T @ W_o_p, PSUM -> SBUF ->
  DRAM.

b_Q/b_K are applied on-device (per-partition bias during the PSUM->SBUF
copy); b_V's exact contribution sum_h W_O[h].T @ b_V[h] (softmax rows sum
to 1) and b_O are added on the host.
"""
import ml_dtypes
import numpy as np

import concourse.bass as bass
import concourse.mybir as mybir
import concourse.tile as tile
from concourse import bacc
from concourse import bass_utils

F32 = mybir.dt.float32
F32R = mybir.dt.float32r
EXPF = mybir.ActivationFunctionType.Exp

S = 2048          # sequence length
DM = 1024         # d_model
DH = 64           # d_head
NHC = 8           # heads per core
PAIRS = 4         # head pairs per core
MC = 8            # d_model chunks of 128
NSB = 4           # query superblocks of 512
SBW = 512         # superblock width
NKT = 16          # key tiles of 128
NST = 16          # seq tiles of 128
MASK_NEG = -1e9
SCALE = 0.125     # 1/sqrt(d_head)

_NC_CACHE = {}
LAST_RESULTS = None


def _build_nc():
    nc = bacc.Bacc("TRN2", target_bir_lowering=False, debug=False)
    BF16 = mybir.dt.bfloat16
    xt_d = nc.dram_tensor("xt", [DM, S], BF16, kind="ExternalInput")
    wq_d = nc.dram_tensor("wq", [PAIRS, MC, 128, 128], BF16, kind="ExternalInput")
    wk_d = nc.dram_tensor("wk", [PAIRS, MC, 128, 128], BF16, kind="ExternalInput")
    wv_d = nc.dram_tensor("wv", [MC, 128, NHC * DH], BF16, kind="ExternalInput")
    wo_d = nc.dram_tensor("wo", [PAIRS, 128, DM], BF16, kind="ExternalInput")
    bq_d = nc.dram_tensor("bq", [PAIRS, 128, 1], F32, kind="ExternalInput")
    bk_d = nc.dram_tensor("bk", [PAIRS, 128, 1], F32, kind="ExternalInput")
    msk_d = nc.dram_tensor("mask", [128, 256], BF16, kind="ExternalInput")
    id_d = nc.dram_tensor("ident", [128, 128], BF16, kind="ExternalInput")
    out_d = nc.dram_tensor("out", [S, DM], F32, kind="ExternalOutput")

    with tile.TileContext(nc) as tc:
      with (
          tc.tile_pool(name="hold", bufs=1) as hold,
          tc.tile_pool(name="ph2", bufs=1) as ph2,
          tc.tile_pool(name="patn", bufs=1, space="PSUM") as patn,
      ):
        v_t = [hold.tile([128, NHC, DH + 1], BF16, tag=f"v{st}", name=f"v{st}") for st in range(NST)]
        z_t = [hold.tile([128, S], BF16, tag=f"z{p}", name=f"z{p}") for p in range(PAIRS)]
        msk_t = hold.tile([128, 256], BF16, tag="mtri")
        id_t = hold.tile([128, 128], BF16, tag="ident")
        bq_t = [hold.tile([128, 1], F32, tag=f"bq{p}", name=f"bq{p}") for p in range(PAIRS)]
        bk_t = [hold.tile([128, 1], F32, tag=f"bk{p}", name=f"bk{p}") for p in range(PAIRS)]
        ones_c = hold.tile([128, 1], F32, tag="ones")
        qts = {}

        nc.vector.memset(ones_c[:], 1.0)
        # small constants go through the (otherwise idle) gpsimd DMA queue so
        # their triggers don't delay the xt/wv bulk loads
        nc.gpsimd.dma_start(msk_t[:], msk_d.ap())
        nc.gpsimd.dma_start(id_t[:], id_d.ap())
        for p in range(PAIRS):
            nc.gpsimd.dma_start(bq_t[p][:], bq_d.ap()[p])
            nc.gpsimd.dma_start(bk_t[p][:], bk_d.ap()[p])

        def attn_j(p, sb, j, z0, z1):
            qt, kt = qts[p]
            qtb = qt[sb]
            ktb = kt[j // 4]
            nkt = 4 * (sb + 1)
            # columns q < j*128 of this key tile are fully masked;
            # restrict S/exp/PV to the valid suffix.
            j_rel = j - 4 * sb
            off = max(j_rel, 0) * 128
            sp = patn.tile([128, 1024], F32, tag="sp", bufs=2, name="sp")
            ks = ((j % 4) * 128, (j % 4 + 1) * 128)
            diag = j_rel >= 0
            nc.tensor.matmul(
                sp[:, off:512],
                ktb[0:64, ks[0]:ks[1]],
                qtb[0:64, off:SBW],
                start=True, stop=not diag,
                tile_position=(0, 0),
                skip_group_check=True,
            )
            nc.tensor.matmul(
                sp[:, 512 + off:1024],
                ktb[64:128, ks[0]:ks[1]],
                qtb[64:128, off:SBW],
                start=True, stop=not diag,
                tile_position=(64, 0),
                skip_group_check=True,
            )
            sp3 = sp[:].rearrange("p (u q) -> p u q", u=2)
            if diag:
                # add the causal triangle to the diagonal block in-PSUM:
                # out += I.T @ mask  (PE accumulate, no DVE on critical path)
                for u in (0, 1):
                    lo = u * 512 + off
                    nc.tensor.matmul(
                        sp[:, lo:lo + 128],
                        id_t[:],
                        msk_t[:, 0:128],
                        start=False, stop=True,
                        skip_group_check=True,
                    )
            pt = ph2.tile([128, 1024], BF16, tag="pt", bufs=6, name="pt")
            pt3 = pt[:].rearrange("p (u q) -> p u q", u=2)
            nc.scalar.activation(
                pt3[:, :, off:512], sp3[:, :, off:512], EXPF, scale=SCALE
            )
            nc.tensor.matmul(
                z0[:, off:512],
                v_t[j][:, 2 * p, :],
                pt[:, off:512],
                start=(j == 0), stop=(j == nkt - 1),
            )
            nc.tensor.matmul(
                z1[:, off:512],
                v_t[j][:, 2 * p + 1, :],
                pt[:, 512 + off:1024],
                start=(j == 0), stop=(j == nkt - 1),
            )

        def attn_norm(p, sb, z0, z1, q0=0, q1=512):
            # normalize by the softmax denominator (row DH of z psum).
            # First copy z psum to SBUF so the bank frees immediately (the
            # next superblock's PV only waits for this copy, not the whole
            # reciprocal/broadcast/multiply chain). Optional [q0,q1) restricts
            # to a column chunk so the tail can pipeline norm with oproj.
            qs = (sb * SBW + q0, sb * SBW + q1)
            w = q1 - q0
            d0row = ph2.tile([1, 512], F32, tag="d0row", bufs=2, name="d0row")
            d1row = ph2.tile([1, 512], F32, tag="d1row", bufs=2, name="d1row")
            nc.vector.tensor_copy(d0row[:, 0:w], z0[DH:DH + 1, q0:q1])
            nc.vector.tensor_copy(d1row[:, 0:w], z1[DH:DH + 1, q0:q1])
            nc.vector.reciprocal_approx_fast(d0row[:, 0:w], d0row[:, 0:w])
            nc.vector.reciprocal_approx_fast(d1row[:, 0:w], d1row[:, 0:w])
            r0 = ph2.tile([64, 512], F32, tag="r0", bufs=2, name="r0")
            r1 = ph2.tile([64, 512], F32, tag="r1", bufs=2, name="r1")
            nc.gpsimd.partition_broadcast(r0[:, 0:w], d0row[:, 0:w], channels=64)
            nc.gpsimd.partition_broadcast(r1[:, 0:w], d1row[:, 0:w], channels=64)
            nc.vector.tensor_mul(z_t[p][0:64, qs[0]:qs[1]], z0[0:64, q0:q1], r0[:, 0:w])
            t1 = ph2.tile([64, 512], BF16, tag="t1", bufs=2, name="t1")
            nc.vector.tensor_mul(t1[:, 0:w], z1[0:64, q0:q1], r1[:, 0:w])
            nc.sync.dma_start(z_t[p][64:128, qs[0]:qs[1]], t1[:, 0:w])

        with (
            tc.tile_pool(name="ph1", bufs=1) as ph1,
            tc.tile_pool(name="pqk", bufs=1, space="PSUM") as pqk,
        ):
            # xt in per-512-column-block tiles: attention(0, sb) needs only
            # Q/K columns <= (sb+1)*512 (causal), so the whole front of the
            # kernel pipelines by column block.
            xt_t = [[ph1.tile([128, SBW], BF16, tag=f"xt{m}_{cb}", name=f"xt{m}_{cb}")
                     for cb in range(NSB)] for m in range(MC)]
            wv_t = [ph1.tile([128, NHC * DH], BF16, tag=f"wv{m}", name=f"wv{m}") for m in range(MC)]

            def qk_gen(p, sb_outer=False):
                """QK projection for pair p (bf16, pair-stacked partitions),
                yielded one matmul at a time for interleaving. With
                sb_outer=True the superblock loop is outermost so early
                superblocks finish as soon as their xt column block lands."""
                qt = [hold.tile([128, SBW], BF16, tag=f"qt{i}", bufs=2, name=f"qt{i}")
                      for i in range(NSB)]
                kt = [hold.tile([128, SBW], BF16, tag=f"kt{i}", bufs=2, name=f"kt{i}")
                      for i in range(NSB)]
                qts[p] = (qt, kt)
                wqk = []
                for (w_d, b_t, dst) in ((wq_d, bq_t, qt), (wk_d, bk_t, kt)):
                    wts = []
                    for m in range(MC):
                        w = ph1.tile([128, 128], BF16, tag="w", bufs=16, name="w")
                        nc.sync.dma_start(w[:], w_d.ap()[p, m])
                        wts.append(w)
                    wqk.append((wts, b_t, dst))
                order = (
                    [(sb, wb) for sb in range(NSB) for wb in wqk]
                    if sb_outer else
                    [(sb, wb) for wb in wqk for sb in range(NSB)]
                )
                for sb, (wts, b_t, dst) in order:
                    ps = pqk.tile([128, 512], F32, tag="acc", bufs=2, name="acc")
                    for m in range(MC):
                        nc.tensor.matmul(
                            ps[:],
                            wts[m][:],
                            xt_t[m][sb][:],
                            start=(m == 0),
                            stop=(m == MC - 1),
                        )
                        yield
                    nc.vector.tensor_scalar_add(dst[sb][:], ps[:], b_t[p][:])
                    yield

            # column-block pipelined prelude: per block, land xt columns,
            # then V-projection for its 4 seq tiles and pair 0's QK for it.
            g0 = qk_gen(0, sb_outer=True)
            for cb in range(NSB):
                for m in range(MC):
                    # alternate trigger queues: each dma_start costs ~0.6us of
                    # issuing-engine queue time, which otherwise serializes
                    eng = nc.scalar if m % 2 == 0 else nc.sync
                    eng.dma_start(
                        xt_t[m][cb][:],
                        xt_d.ap()[m * 128:(m + 1) * 128, cb * SBW:(cb + 1) * SBW],
                    )
                    if cb == 0:
                        eng2 = nc.sync if m % 2 == 0 else nc.scalar
                        eng2.dma_start(wv_t[m][:], wv_d.ap()[m])
                for st in range(4 * cb, 4 * cb + 4):
                    ps = pqk.tile([128, 512], F32, tag="acc", bufs=2, name="acc")
                    for m in range(MC):
                        nc.tensor.matmul(
                            ps[:],
                            xt_t[m][cb][:, (st % 4) * 128:(st % 4 + 1) * 128],
                            wv_t[m][:],
                            start=(m == 0),
                            stop=(m == MC - 1),
                        )
                    nc.vector.tensor_copy(
                        v_t[st][:, :, 0:DH],
                        ps[:].rearrange("p (h d) -> p h d", h=NHC),
                    )
                    nc.vector.tensor_copy(
                        v_t[st][:, :, DH],
                        ones_c[:].to_broadcast((128, NHC)),
                    )
                for _ in range(18):  # one QK column-block (2 proj x (8 mm + copy))
                    try:
                        next(g0)
                    except StopIteration:
                        break
            for _ in g0:
                pass

            # attention for pairs 0-2, with pair p+1's projection matmuls
            # interleaved into the ACT-paced attention stream
            for p in range(3):
                g = qk_gen(p + 1)
                done = False
                emitted = 0
                step = 0
                for sb in range(NSB):
                    nkt = 4 * (sb + 1)
                    z0 = patn.tile([DH + 1, 512], F32, tag="z0", bufs=1, name="z0")
                    z1 = patn.tile([DH + 1, 512], F32, tag="z1", bufs=1, name="z1")
                    for j in range(nkt):
                        attn_j(p, sb, j, z0, z1)
                        step += 1
                        want = 2 * step if step <= 32 else 64 + (step - 32)
                        while emitted < want and not done:
                            try:
                                next(g)
                                emitted += 1
                            except StopIteration:
                                done = True
                    attn_norm(p, sb, z0, z1)
                while not done:
                    try:
                        next(g)
                    except StopIteration:
                        done = True

        # ---------------- last pair + output projection ----------------
        with tc.tile_pool(name="ph3", bufs=1) as ph3:
            wo_t = [ph3.tile([128, DM], BF16, tag=f"wo{p}", name=f"wo{p}") for p in range(PAIRS)]
            for p in range(PAIRS):
                nc.sync.dma_start(wo_t[p][:], wo_d.ap()[p])

            def oproj(q, mb, tag="sp", cp=None):
                ps = patn.tile([128, 512], F32, tag=tag, bufs=2 if tag == "sp" else 1, name="ops")
                for p in range(PAIRS):
                    nc.tensor.matmul(
                        ps[:],
                        z_t[p][:, q * 128:(q + 1) * 128],
                        wo_t[p][:, mb * 512:(mb + 1) * 512],
                        start=(p == 0),
                        stop=(p == PAIRS - 1),
                    )
                ost = ph3.tile([128, 512], F32, tag="ost", bufs=4, name="ost")
                (cp or nc.vector.tensor_copy)(ost[:], ps[:])
                nc.sync.dma_start(
                    out_d.ap()[q * 128:(q + 1) * 128, mb * 512:(mb + 1) * 512],
                    ost[:],
                )

            # pair 3's attention, with output-projection tiles for already-
            # complete superblocks interleaved in (sb lags by one).
            otodo = [(q, mb) for q in range(NST) for mb in range(2)]
            odone = 0
            # alternate z psum between two tag sets (the second lives in the
            # banks freed by the projection pool) so superblock boundaries
            # don't stall on the previous normalize.
            for sb in range(NSB):
                nkt = 4 * (sb + 1)
                z0 = patn.tile([DH + 1, 512], F32, tag="z0", bufs=1, name="z0")
                z1 = patn.tile([DH + 1, 512], F32, tag="z1", bufs=1, name="z1")
                for j in range(nkt):
                    attn_j(3, sb, j, z0, z1)
                    # z for superblocks < sb is final for all pairs; issue up
                    # to 2 units per step so the backlog drains before the end
                    ready = sb * 8
                    for _ in range(2):
                        if odone < ready:
                            oproj(*otodo[odone])
                            odone += 1
                attn_norm(3, sb, z0, z1)
            # final groups (need the last superblock's z): emit p-major in
            # waves of 4 so the pair-0..2 matmuls run during the attention
            # tail and only the final per-group matmul waits on the last
            # normalize; copies alternate between the two idle engines.
            rest = otodo[odone:]
            for w0 in range(0, len(rest), 4):
                wave = rest[w0:w0 + 4]
                tags = ["sp", "sp", "z0", "z1"]
                pss = [
                    patn.tile([128, 512], F32, tag=tags[i],
                              bufs=2 if tags[i] == "sp" else 1, name="opsf")
                    for i in range(len(wave))
                ]
                for p in range(PAIRS):
                    for (q, mb), ps in zip(wave, pss):
                        nc.tensor.matmul(
                            ps[:],
                            z_t[p][:, q * 128:(q + 1) * 128],
                            wo_t[p][:, mb * 512:(mb + 1) * 512],
                            start=(p == 0),
                            stop=(p == PAIRS - 1),
                            skip_group_check=True,
                        )
                for i, ((q, mb), ps) in enumerate(zip(wave, pss)):
                    ost = ph3.tile([128, 512], F32, tag="ost", bufs=4, name="ost")
                    cp = nc.scalar.copy if i % 2 == 0 else nc.vector.tensor_copy
                    cp(ost[:], ps[:])
                    nc.sync.dma_start(
                        out_d.ap()[q * 128:(q + 1) * 128, mb * 512:(mb + 1) * 512],
                        ost[:],
                    )

    nc.compile()
    return nc


def _get_nc():
    if "nc" not in _NC_CACHE:
        _NC_CACHE["nc"] = _build_nc()
    return _NC_CACHE["nc"]


def _causal_masks():
    k = np.arange(128)[:, None]
    q = np.arange(128)[None, :]
    tri = np.where(q >= k, 0.0, MASK_NEG).astype(ml_dtypes.bfloat16)
    return np.concatenate([tri, tri], axis=1)  # one copy per head half


def kernel(resid_pre, W_Q, W_K, W_V, W_O, b_Q, b_K, b_V, b_O):
    global LAST_RESULTS
    resid_pre = np.asarray(resid_pre, dtype=np.float32)
    W_Q = np.asarray(W_Q, dtype=np.float32)
    W_K = np.asarray(W_K, dtype=np.float32)
    W_V = np.asarray(W_V, dtype=np.float32)
    W_O = np.asarray(W_O, dtype=np.float32)
    b_Q = np.asarray(b_Q, dtype=np.float32)
    b_K = np.asarray(b_K, dtype=np.float32)
    b_V = np.asarray(b_V, dtype=np.float32)
    b_O = np.asarray(b_O, dtype=np.float32)

    B = resid_pre.shape[0]
    masks = _causal_masks()
    ident = np.eye(128, dtype=ml_dtypes.bfloat16)

    def pack_pairs(w):  # [8, 1024, 64] -> [4, 8, 128, 128]
        return np.ascontiguousarray(
            w.reshape(PAIRS, 2, DM, DH).transpose(0, 2, 1, 3).reshape(PAIRS, MC, 128, 128)
        )

    in_maps = []
    for c in range(8):
        b, g = divmod(c, 2)
        hs = slice(g * NHC, (g + 1) * NHC)
        in_maps.append({
            "xt": np.ascontiguousarray(resid_pre[b].T).astype(ml_dtypes.bfloat16),
            "wq": pack_pairs(W_Q[hs]).astype(ml_dtypes.bfloat16),
            "wk": pack_pairs(W_K[hs]).astype(ml_dtypes.bfloat16),
            "wv": np.ascontiguousarray(
                W_V[hs].transpose(1, 0, 2).reshape(DM, NHC * DH).reshape(MC, 128, NHC * DH)
            ).astype(ml_dtypes.bfloat16),
            "wo": np.ascontiguousarray(W_O[hs].reshape(PAIRS, 128, DM)).astype(ml_dtypes.bfloat16),
            "bq": np.ascontiguousarray(b_Q[hs].reshape(PAIRS, 128, 1)),
            "bk": np.ascontiguousarray(b_K[hs].reshape(PAIRS, 128, 1)),
            "mask": masks,
            "ident": ident,
        })

    nc = _get_nc()
    res = bass_utils.run_bass_kernel_spmd(nc, in_maps, core_ids=list(range(8)))
    LAST_RESULTS = res

    # b_V contributes exactly sum_h W_O[h].T @ b_V[h] (softmax rows sum to 1)
    const = np.einsum("hdm,hd->m", W_O, b_V).astype(np.float32) + b_O
    out = np.empty((B, S, DM), dtype=np.float32)
    for b in range(B):
        out[b] = res.results[2 * b]["out"] + res.results[2 * b + 1]["out"] + const
    return out



# revision 16
# speedup vs baseline: 1.2287x; 1.0644x over previous
"""Multi-head causal attention on 8 Trainium2 NeuronCores.

Problem: resid_pre [4, 2048, 1024], 16 heads x d_head 64, causal softmax,
output [4, 2048, 1024] f32.

Sharding: data-parallel over the 4 batches x tensor-parallel over 2 head
groups (8 heads each) -> 8 cores. Each core computes the attention output
contribution of its 8 heads for its batch; the host sums the two head-group
partials per batch (the "all-reduce") and adds the output bias.

Per-core kernel (matmul inputs bf16, all accumulation fp32 in PSUM;
measured ~3.5e-3 max rel err vs the fp32 reference):

  prelude, pipelined by 512-column blocks of X^T (causality means attention
  superblock sb only needs Q/K columns <= (sb+1)*512):
    V = X @ W_v for all 8 heads in natural [seq, d] layout with a ones
    column appended per head, and Q^T/K^T for head pair 0, pair-stacked on
    partitions (head 2p in partitions 0-63, 2p+1 in 64-127).

  per head pair p (heads 2p, 2p+1), per 512-wide query superblock, per
  128-wide key tile:
    S^T = K^T.T @ Q^T (keys on partitions, one matmul per head via
    partition row groups), restricted to the un-masked column suffix;
    causal triangle added to the diagonal block in-PSUM via an
    identity-stationary matmul; exp on ScalarE (no max subtraction needed,
    scores are O(1)); z~^T[65, 512] += V_chunk.T @ P~^T accumulated in
    PSUM, whose row 64 (from the ones column) is the softmax denominator;
    normalize with reciprocal_approx_fast + gpsimd partition broadcast.
    Pair p+1's Q/K projection matmuls are interleaved into this ACT-paced
    stream so the PE never starves; for the last pair the output
    projection tiles of already-final superblocks are interleaved instead.

  output projection: out[q, m] = sum_p z^T_p.T @ W_o_p, PSUM -> SBUF ->
  DRAM.

b_Q/b_K are applied on-device (per-partition bias during the PSUM->SBUF
copy); b_V's exact contribution sum_h W_O[h].T @ b_V[h] (softmax rows sum
to 1) and b_O are added on the host.
"""
import ml_dtypes
import numpy as np

import concourse.bass as bass
import concourse.mybir as mybir
import concourse.tile as tile
from concourse import bacc
from concourse import bass_utils

F32 = mybir.dt.float32
F32R = mybir.dt.float32r
EXPF = mybir.ActivationFunctionType.Exp

S = 2048          # sequence length
DM = 1024         # d_model
DH = 64           # d_head
NHC = 8           # heads per core
PAIRS = 4         # head pairs per core
MC = 8            # d_model chunks of 128
NSB = 4           # query superblocks of 512
SBW = 512         # superblock width
NKT = 16          # key tiles of 128
NST = 16          # seq tiles of 128
MASK_NEG = -1e9
SCALE = 0.125     # 1/sqrt(d_head)

_NC_CACHE = {}
LAST_RESULTS = None


def _build_nc():
    nc = bacc.Bacc("TRN2", target_bir_lowering=False, debug=False)
    BF16 = mybir.dt.bfloat16
    xt_d = nc.dram_tensor("xt", [DM, S], BF16, kind="ExternalInput")
    wq_d = nc.dram_tensor("wq", [PAIRS, MC, 128, 128], BF16, kind="ExternalInput")
    wk_d = nc.dram_tensor("wk", [PAIRS, MC, 128, 128], BF16, kind="ExternalInput")
    wv_d = nc.dram_tensor("wv", [MC, 128, NHC * DH], BF16, kind="ExternalInput")
    wo_d = nc.dram_tensor("wo", [PAIRS, 128, DM], BF16, kind="ExternalInput")
    bq_d = nc.dram_tensor("bq", [PAIRS, 128, 1], F32, kind="ExternalInput")
    bk_d = nc.dram_tensor("bk", [PAIRS, 128, 1], F32, kind="ExternalInput")
    msk_d = nc.dram_tensor("mask", [128, 256], BF16, kind="ExternalInput")
    id_d = nc.dram_tensor("ident", [128, 128], BF16, kind="ExternalInput")
    out_d = nc.dram_tensor("out", [S, DM], F32, kind="ExternalOutput")

    with tile.TileContext(nc) as tc:
      with (
          tc.tile_pool(name="hold", bufs=1) as hold,
          tc.tile_pool(name="ph2", bufs=1) as ph2,
          tc.tile_pool(name="patn", bufs=1, space="PSUM") as patn,
      ):
        v_t = [hold.tile([128, NHC, DH + 1], BF16, tag=f"v{st}", name=f"v{st}") for st in range(NST)]
        z_t = [hold.tile([128, S], BF16, tag=f"z{p}", name=f"z{p}") for p in range(PAIRS)]
        msk_t = hold.tile([128, 256], BF16, tag="mtri")
        id_t = hold.tile([128, 128], BF16, tag="ident")
        bq_t = [hold.tile([128, 1], F32, tag=f"bq{p}", name=f"bq{p}") for p in range(PAIRS)]
        bk_t = [hold.tile([128, 1], F32, tag=f"bk{p}", name=f"bk{p}") for p in range(PAIRS)]
        ones_c = hold.tile([128, 1], F32, tag="ones")
        qts = {}

        nc.vector.memset(ones_c[:], 1.0)
        # small constants go through the (otherwise idle) gpsimd DMA queue so
        # their triggers don't delay the xt/wv bulk loads
        nc.gpsimd.dma_start(msk_t[:], msk_d.ap())
        nc.gpsimd.dma_start(id_t[:], id_d.ap())
        for p in range(PAIRS):
            nc.gpsimd.dma_start(bq_t[p][:], bq_d.ap()[p])
            nc.gpsimd.dma_start(bk_t[p][:], bk_d.ap()[p])

        def attn_j(p, sb, j, z0, z1):
            qt, kt = qts[p]
            qtb = qt[sb]
            ktb = kt[j // 4]
            nkt = 4 * (sb + 1)
            # columns q < j*128 of this key tile are fully masked;
            # restrict S/exp/PV to the valid suffix.
            j_rel = j - 4 * sb
            off = max(j_rel, 0) * 128
            sp = patn.tile([128, 1024], F32, tag="sp", bufs=2, name="sp")
            ks = ((j % 4) * 128, (j % 4 + 1) * 128)
            diag = j_rel >= 0
            nc.tensor.matmul(
                sp[:, off:512],
                ktb[0:64, ks[0]:ks[1]],
                qtb[0:64, off:SBW],
                start=True, stop=not diag,
                tile_position=(0, 0),
                skip_group_check=True,
            )
            nc.tensor.matmul(
                sp[:, 512 + off:1024],
                ktb[64:128, ks[0]:ks[1]],
                qtb[64:128, off:SBW],
                start=True, stop=not diag,
                tile_position=(64, 0),
                skip_group_check=True,
            )
            sp3 = sp[:].rearrange("p (u q) -> p u q", u=2)
            if diag:
                # add the causal triangle to the diagonal block in-PSUM:
                # out += I.T @ mask  (PE accumulate, no DVE on critical path)
                for u in (0, 1):
                    lo = u * 512 + off
                    nc.tensor.matmul(
                        sp[:, lo:lo + 128],
                        id_t[:],
                        msk_t[:, 0:128],
                        start=False, stop=True,
                        skip_group_check=True,
                    )
            pt = ph2.tile([128, 1024], BF16, tag="pt", bufs=6, name="pt")
            pt3 = pt[:].rearrange("p (u q) -> p u q", u=2)
            nc.scalar.activation(
                pt3[:, :, off:512], sp3[:, :, off:512], EXPF, scale=SCALE
            )
            nc.tensor.matmul(
                z0[:, off:512],
                v_t[j][:, 2 * p, :],
                pt[:, off:512],
                start=(j == 0), stop=(j == nkt - 1),
            )
            nc.tensor.matmul(
                z1[:, off:512],
                v_t[j][:, 2 * p + 1, :],
                pt[:, 512 + off:1024],
                start=(j == 0), stop=(j == nkt - 1),
            )

        def attn_norm(p, sb, z0, z1, q0=0, q1=512):
            # normalize by the softmax denominator (row DH of z psum).
            # First copy z psum to SBUF so the bank frees immediately (the
            # next superblock's PV only waits for this copy, not the whole
            # reciprocal/broadcast/multiply chain). Optional [q0,q1) restricts
            # to a column chunk so the tail can pipeline norm with oproj.
            qs = (sb * SBW + q0, sb * SBW + q1)
            w = q1 - q0
            d0row = ph2.tile([1, 512], F32, tag="d0row", bufs=2, name="d0row")
            d1row = ph2.tile([1, 512], F32, tag="d1row", bufs=2, name="d1row")
            nc.vector.tensor_copy(d0row[:, 0:w], z0[DH:DH + 1, q0:q1])
            nc.vector.tensor_copy(d1row[:, 0:w], z1[DH:DH + 1, q0:q1])
            nc.vector.reciprocal_approx_fast(d0row[:, 0:w], d0row[:, 0:w])
            nc.vector.reciprocal_approx_fast(d1row[:, 0:w], d1row[:, 0:w])
            r0 = ph2.tile([64, 512], F32, tag="r0", bufs=2, name="r0")
            r1 = ph2.tile([64, 512], F32, tag="r1", bufs=2, name="r1")
            nc.gpsimd.partition_broadcast(r0[:, 0:w], d0row[:, 0:w], channels=64)
            nc.gpsimd.partition_broadcast(r1[:, 0:w], d1row[:, 0:w], channels=64)
            nc.vector.tensor_mul(z_t[p][0:64, qs[0]:qs[1]], z0[0:64, q0:q1], r0[:, 0:w])
            t1 = ph2.tile([64, 512], BF16, tag="t1", bufs=2, name="t1")
            nc.vector.tensor_mul(t1[:, 0:w], z1[0:64, q0:q1], r1[:, 0:w])
            nc.sync.dma_start(z_t[p][64:128, qs[0]:qs[1]], t1[:, 0:w])

        with (
            tc.tile_pool(name="ph1", bufs=1) as ph1,
            tc.tile_pool(name="pqk", bufs=1, space="PSUM") as pqk,
        ):
            # xt in per-512-column-block tiles: attention(0, sb) needs only
            # Q/K columns <= (sb+1)*512 (causal), so the whole front of the
            # kernel pipelines by column block.
            xt_t = [[ph1.tile([128, SBW], BF16, tag=f"xt{m}_{cb}", name=f"xt{m}_{cb}")
                     for cb in range(NSB)] for m in range(MC)]
            wv_t = [ph1.tile([128, NHC * DH], BF16, tag=f"wv{m}", name=f"wv{m}") for m in range(MC)]

            def qk_gen(p, sb_outer=False):
                """QK projection for pair p (bf16, pair-stacked partitions),
                yielded one matmul at a time for interleaving. With
                sb_outer=True the superblock loop is outermost so early
                superblocks finish as soon as their xt column block lands."""
                qt = [hold.tile([128, SBW], BF16, tag=f"qt{i}", bufs=2, name=f"qt{i}")
                      for i in range(NSB)]
                kt = [hold.tile([128, SBW], BF16, tag=f"kt{i}", bufs=2, name=f"kt{i}")
                      for i in range(NSB)]
                qts[p] = (qt, kt)
                wqk = []
                for (w_d, b_t, dst) in ((wq_d, bq_t, qt), (wk_d, bk_t, kt)):
                    wts = []
                    for m in range(MC):
                        w = ph1.tile([128, 128], BF16, tag="w", bufs=16, name="w")
                        nc.sync.dma_start(w[:], w_d.ap()[p, m])
                        wts.append(w)
                    wqk.append((wts, b_t, dst))
                order = (
                    [(sb, wb) for sb in range(NSB) for wb in wqk]
                    if sb_outer else
                    [(sb, wb) for wb in wqk for sb in range(NSB)]
                )
                for sb, (wts, b_t, dst) in order:
                    ps = pqk.tile([128, 512], F32, tag="acc", bufs=2, name="acc")
                    for m in range(MC):
                        nc.tensor.matmul(
                            ps[:],
                            wts[m][:],
                            xt_t[m][sb][:],
                            start=(m == 0),
                            stop=(m == MC - 1),
                        )
                        yield
                    nc.vector.tensor_scalar_add(dst[sb][:], ps[:], b_t[p][:])
                    yield

            # column-block pipelined prelude: per block, land xt columns,
            # then V-projection for its 4 seq tiles and pair 0's QK for it.
            g0 = qk_gen(0, sb_outer=True)
            for cb in range(NSB):
                for m in range(MC):
                    # alternate trigger queues: each dma_start costs ~0.6us of
                    # issuing-engine queue time, which otherwise serializes
                    eng = nc.scalar if m % 2 == 0 else nc.sync
                    eng.dma_start(
                        xt_t[m][cb][:],
                        xt_d.ap()[m * 128:(m + 1) * 128, cb * SBW:(cb + 1) * SBW],
                    )
                    if cb == 0:
                        eng2 = nc.sync if m % 2 == 0 else nc.scalar
                        eng2.dma_start(wv_t[m][:], wv_d.ap()[m])
                for st in range(4 * cb, 4 * cb + 4):
                    ps = pqk.tile([128, 512], F32, tag="acc", bufs=2, name="acc")
                    for m in range(MC):
                        nc.tensor.matmul(
                            ps[:],
                            xt_t[m][cb][:, (st % 4) * 128:(st % 4 + 1) * 128],
                            wv_t[m][:],
                            start=(m == 0),
                            stop=(m == MC - 1),
                        )
                    nc.vector.tensor_copy(
                        v_t[st][:, :, 0:DH],
                        ps[:].rearrange("p (h d) -> p h d", h=NHC),
                    )
                    nc.vector.tensor_copy(
                        v_t[st][:, :, DH],
                        ones_c[:].to_broadcast((128, NHC)),
                    )
                for _ in range(18):  # one QK column-block (2 proj x (8 mm + copy))
                    try:
                        next(g0)
                    except StopIteration:
                        break
            for _ in g0:
                pass

            # attention for pairs 0-2, with pair p+1's projection matmuls
            # interleaved into the ACT-paced attention stream
            for p in range(3):
                g = qk_gen(p + 1)
                done = False
                emitted = 0
                step = 0
                for sb in range(NSB):
                    nkt = 4 * (sb + 1)
                    z0 = patn.tile([DH + 1, 512], F32, tag="z0", bufs=1, name="z0")
                    z1 = patn.tile([DH + 1, 512], F32, tag="z1", bufs=1, name="z1")
                    for j in range(nkt):
                        attn_j(p, sb, j, z0, z1)
                        step += 1
                        want = 2 * step if step <= 32 else 64 + (step - 32)
                        while emitted < want and not done:
                            try:
                                next(g)
                                emitted += 1
                            except StopIteration:
                                done = True
                    attn_norm(p, sb, z0, z1)
                while not done:
                    try:
                        next(g)
                    except StopIteration:
                        done = True

        # ---------------- last pair + output projection ----------------
        with (
            tc.tile_pool(name="ph3", bufs=1) as ph3,
            tc.tile_pool(name="po", bufs=1, space="PSUM") as po,
        ):
            wo_t = [ph3.tile([128, DM], BF16, tag=f"wo{p}", name=f"wo{p}") for p in range(PAIRS)]
            for p in range(PAIRS):
                nc.sync.dma_start(wo_t[p][:], wo_d.ap()[p])

            def oproj(q, mb, cp=None):
                # own psum pool (the banks freed by the closed pqk pool) so
                # units never contend with pair-3's score psum
                ps = po.tile([128, 512], F32, tag="oacc", bufs=2, name="ops")
                for p in range(PAIRS):
                    nc.tensor.matmul(
                        ps[:],
                        z_t[p][:, q * 128:(q + 1) * 128],
                        wo_t[p][:, mb * 512:(mb + 1) * 512],
                        start=(p == 0),
                        stop=(p == PAIRS - 1),
                    )
                ost = ph3.tile([128, 512], F32, tag="ost", bufs=4, name="ost")
                (cp or nc.vector.tensor_copy)(ost[:], ps[:])
                nc.sync.dma_start(
                    out_d.ap()[q * 128:(q + 1) * 128, mb * 512:(mb + 1) * 512],
                    ost[:],
                )

            # pair 3's attention, with output-projection tiles for already-
            # complete superblocks interleaved in (sb lags by one).
            otodo = [(q, mb) for q in range(NST) for mb in range(2)]
            odone = 0
            # alternate z psum between two tag sets (the second lives in the
            # banks freed by the projection pool) so superblock boundaries
            # don't stall on the previous normalize.
            for sb in range(NSB):
                nkt = 4 * (sb + 1)
                z0 = patn.tile([DH + 1, 512], F32, tag="z0", bufs=1, name="z0")
                z1 = patn.tile([DH + 1, 512], F32, tag="z1", bufs=1, name="z1")
                for j in range(nkt):
                    attn_j(3, sb, j, z0, z1)
                    # z for superblocks < sb is final for all pairs; issue up
                    # to 2 units per step so the backlog drains before the end
                    ready = sb * 8
                    for _ in range(2):
                        if odone < ready:
                            oproj(*otodo[odone])
                            odone += 1
                if sb < NSB - 1:
                    attn_norm(3, sb, z0, z1)
                else:
                    # last superblock: chunk the normalize by 128 columns and
                    # emit each q-tile's 2 output units right after its chunk,
                    # so the norm chain pipelines with the final projections
                    for c in range(4):
                        attn_norm(3, sb, z0, z1, q0=c * 128, q1=(c + 1) * 128)
                        if c == 0:
                            # drain any interleave backlog (needs only sb<3 z,
                            # which is final) while chunk 0's norm chain runs
                            while odone < 24:
                                oproj(*otodo[odone])
                                odone += 1
                        for i in range(2):
                            oproj(*otodo[odone],
                                  cp=(nc.scalar.copy if i % 2 == 0 else None))
                            odone += 1

    nc.compile()
    return nc


def _get_nc():
    if "nc" not in _NC_CACHE:
        _NC_CACHE["nc"] = _build_nc()
    return _NC_CACHE["nc"]


def _causal_masks():
    k = np.arange(128)[:, None]
    q = np.arange(128)[None, :]
    tri = np.where(q >= k, 0.0, MASK_NEG).astype(ml_dtypes.bfloat16)
    return np.concatenate([tri, tri], axis=1)  # one copy per head half


def kernel(resid_pre, W_Q, W_K, W_V, W_O, b_Q, b_K, b_V, b_O):
    global LAST_RESULTS
    resid_pre = np.asarray(resid_pre, dtype=np.float32)
    W_Q = np.asarray(W_Q, dtype=np.float32)
    W_K = np.asarray(W_K, dtype=np.float32)
    W_V = np.asarray(W_V, dtype=np.float32)
    W_O = np.asarray(W_O, dtype=np.float32)
    b_Q = np.asarray(b_Q, dtype=np.float32)
    b_K = np.asarray(b_K, dtype=np.float32)
    b_V = np.asarray(b_V, dtype=np.float32)
    b_O = np.asarray(b_O, dtype=np.float32)

    B = resid_pre.shape[0]
    masks = _causal_masks()
    ident = np.eye(128, dtype=ml_dtypes.bfloat16)

    def pack_pairs(w):  # [8, 1024, 64] -> [4, 8, 128, 128]
        return np.ascontiguousarray(
            w.reshape(PAIRS, 2, DM, DH).transpose(0, 2, 1, 3).reshape(PAIRS, MC, 128, 128)
        )

    in_maps = []
    for c in range(8):
        b, g = divmod(c, 2)
        hs = slice(g * NHC, (g + 1) * NHC)
        in_maps.append({
            "xt": np.ascontiguousarray(resid_pre[b].T).astype(ml_dtypes.bfloat16),
            "wq": pack_pairs(W_Q[hs]).astype(ml_dtypes.bfloat16),
            "wk": pack_pairs(W_K[hs]).astype(ml_dtypes.bfloat16),
            "wv": np.ascontiguousarray(
                W_V[hs].transpose(1, 0, 2).reshape(DM, NHC * DH).reshape(MC, 128, NHC * DH)
            ).astype(ml_dtypes.bfloat16),
            "wo": np.ascontiguousarray(W_O[hs].reshape(PAIRS, 128, DM)).astype(ml_dtypes.bfloat16),
            "bq": np.ascontiguousarray(b_Q[hs].reshape(PAIRS, 128, 1)),
            "bk": np.ascontiguousarray(b_K[hs].reshape(PAIRS, 128, 1)),
            "mask": masks,
            "ident": ident,
        })

    nc = _get_nc()
    res = bass_utils.run_bass_kernel_spmd(nc, in_maps, core_ids=list(range(8)))
    LAST_RESULTS = res

    # b_V contributes exactly sum_h W_O[h].T @ b_V[h] (softmax rows sum to 1)
    const = np.einsum("hdm,hd->m", W_O, b_V).astype(np.float32) + b_O
    out = np.empty((B, S, DM), dtype=np.float32)
    for b in range(B):
        out[b] = res.results[2 * b]["out"] + res.results[2 * b + 1]["out"] + const
    return out



# revision 19
# speedup vs baseline: 1.2379x; 1.0075x over previous
"""Multi-head causal attention on 8 Trainium2 NeuronCores.

Problem: resid_pre [4, 2048, 1024], 16 heads x d_head 64, causal softmax,
output [4, 2048, 1024] f32.

Sharding: data-parallel over the 4 batches x tensor-parallel over 2 head
groups (8 heads each) -> 8 cores. Each core computes the attention output
contribution of its 8 heads for its batch; the host sums the two head-group
partials per batch (the "all-reduce") and adds the output bias.

Per-core kernel (matmul inputs bf16, all accumulation fp32 in PSUM;
measured ~3.5e-3 max rel err vs the fp32 reference):

  prelude, pipelined by 512-column blocks of X^T (causality means attention
  superblock sb only needs Q/K columns <= (sb+1)*512):
    V = X @ W_v for all 8 heads in natural [seq, d] layout with a ones
    column appended per head, and Q^T/K^T for head pair 0, pair-stacked on
    partitions (head 2p in partitions 0-63, 2p+1 in 64-127).

  per head pair p (heads 2p, 2p+1), per 512-wide query superblock, per
  128-wide key tile:
    S^T = K^T.T @ Q^T (keys on partitions, one matmul per head via
    partition row groups), restricted to the un-masked column suffix;
    causal triangle added to the diagonal block in-PSUM via an
    identity-stationary matmul; exp on ScalarE (no max subtraction needed,
    scores are O(1)); z~^T[65, 512] += V_chunk.T @ P~^T accumulated in
    PSUM, whose row 64 (from the ones column) is the softmax denominator;
    normalize with reciprocal_approx_fast + gpsimd partition broadcast.
    Pair p+1's Q/K projection matmuls are interleaved into this ACT-paced
    stream so the PE never starves; for the last pair the output
    projection tiles of already-final superblocks are interleaved instead.

  output projection: out[q, m] = sum_p z^T_p.T @ W_o_p, PSUM -> SBUF ->
  DRAM.

b_Q/b_K are applied on-device (per-partition bias during the PSUM->SBUF
copy); b_V's exact contribution sum_h W_O[h].T @ b_V[h] (softmax rows sum
to 1) and b_O are added on the host.
"""
import ml_dtypes
import numpy as np

import concourse.bass as bass
import concourse.mybir as mybir
import concourse.tile as tile
from concourse import bacc
from concourse import bass_utils

F32 = mybir.dt.float32
F32R = mybir.dt.float32r
EXPF = mybir.ActivationFunctionType.Exp

S = 2048          # sequence length
DM = 1024         # d_model
DH = 64           # d_head
NHC = 8           # heads per core
PAIRS = 4         # head pairs per core
MC = 8            # d_model chunks of 128
NSB = 4           # query superblocks of 512
SBW = 512         # superblock width
NKT = 16          # key tiles of 128
NST = 16          # seq tiles of 128
MASK_NEG = -1e9
SCALE = 0.125     # 1/sqrt(d_head)

_NC_CACHE = {}
LAST_RESULTS = None


def _build_nc():
    nc = bacc.Bacc("TRN2", target_bir_lowering=False, debug=False)
    BF16 = mybir.dt.bfloat16
    xt_d = nc.dram_tensor("xt", [DM, S], BF16, kind="ExternalInput")
    wq_d = nc.dram_tensor("wq", [PAIRS, MC, 128, 128], BF16, kind="ExternalInput")
    wk_d = nc.dram_tensor("wk", [PAIRS, MC, 128, 128], BF16, kind="ExternalInput")
    wv_d = nc.dram_tensor("wv", [MC, 128, NHC * DH], BF16, kind="ExternalInput")
    wo_d = nc.dram_tensor("wo", [PAIRS, 128, DM], BF16, kind="ExternalInput")
    bq_d = nc.dram_tensor("bq", [PAIRS, 128, 1], F32, kind="ExternalInput")
    bk_d = nc.dram_tensor("bk", [PAIRS, 128, 1], F32, kind="ExternalInput")
    msk_d = nc.dram_tensor("mask", [128, 256], BF16, kind="ExternalInput")
    id_d = nc.dram_tensor("ident", [128, 128], BF16, kind="ExternalInput")
    out_d = nc.dram_tensor("out", [S, DM], F32, kind="ExternalOutput")

    with tile.TileContext(nc) as tc:
      with (
          tc.tile_pool(name="hold", bufs=1) as hold,
          tc.tile_pool(name="ph2", bufs=1) as ph2,
          tc.tile_pool(name="patn", bufs=1, space="PSUM") as patn,
      ):
        v_t = [hold.tile([128, NHC, DH + 1], BF16, tag=f"v{st}", name=f"v{st}") for st in range(NST)]
        z_t = [hold.tile([128, S], BF16, tag=f"z{p}", name=f"z{p}") for p in range(PAIRS)]
        msk_t = hold.tile([128, 256], BF16, tag="mtri")
        id_t = hold.tile([128, 128], BF16, tag="ident")
        bq_t = [hold.tile([128, 1], F32, tag=f"bq{p}", name=f"bq{p}") for p in range(PAIRS)]
        bk_t = [hold.tile([128, 1], F32, tag=f"bk{p}", name=f"bk{p}") for p in range(PAIRS)]
        ones_c = hold.tile([128, 1], F32, tag="ones")
        qts = {}

        nc.vector.memset(ones_c[:], 1.0)
        # small constants go through the (otherwise idle) gpsimd DMA queue so
        # their triggers don't delay the xt/wv bulk loads
        nc.gpsimd.dma_start(msk_t[:], msk_d.ap())
        nc.gpsimd.dma_start(id_t[:], id_d.ap())
        for p in range(PAIRS):
            nc.gpsimd.dma_start(bq_t[p][:], bq_d.ap()[p])
            nc.gpsimd.dma_start(bk_t[p][:], bk_d.ap()[p])

        def attn_scores(p, sb, j):
            """Scores + mask + exp for step j; PV is issued separately so
            filler matmuls can sit between them in the PE's strict FIFO,
            covering the exp latency."""
            qt, kt = qts[p]
            qtb = qt[sb]
            ktb = kt[j // 4]
            # columns q < j*128 of this key tile are fully masked;
            # restrict S/exp/PV to the valid suffix.
            j_rel = j - 4 * sb
            off = max(j_rel, 0) * 128
            sp = patn.tile([128, 1024], F32, tag="sp", bufs=2, name="sp")
            ks = ((j % 4) * 128, (j % 4 + 1) * 128)
            diag = j_rel >= 0
            nc.tensor.matmul(
                sp[:, off:512],
                ktb[0:64, ks[0]:ks[1]],
                qtb[0:64, off:SBW],
                start=True, stop=not diag,
                tile_position=(0, 0),
                skip_group_check=True,
            )
            nc.tensor.matmul(
                sp[:, 512 + off:1024],
                ktb[64:128, ks[0]:ks[1]],
                qtb[64:128, off:SBW],
                start=True, stop=not diag,
                tile_position=(64, 0),
                skip_group_check=True,
            )
            if diag:
                # add the causal triangle to the diagonal block in-PSUM:
                # out += I.T @ mask  (PE accumulate, no DVE on critical path)
                for u in (0, 1):
                    lo = u * 512 + off
                    nc.tensor.matmul(
                        sp[:, lo:lo + 128],
                        id_t[:],
                        msk_t[:, 0:128],
                        start=False, stop=True,
                        skip_group_check=True,
                    )
            pt = ph2.tile([128, 1024], BF16, tag="pt", bufs=6, name="pt")
            if off == 0:
                # contiguous suffix: flat 2D AP is slightly cheaper on ACT
                nc.scalar.activation(pt[:, 0:1024], sp[:, 0:1024], EXPF, scale=SCALE)
            else:
                sp3 = sp[:].rearrange("p (u q) -> p u q", u=2)
                pt3 = pt[:].rearrange("p (u q) -> p u q", u=2)
                nc.scalar.activation(
                    pt3[:, :, off:512], sp3[:, :, off:512], EXPF, scale=SCALE
                )
            return pt, off

        def attn_pv(p, sb, j, z0, z1, pt, off):
            nkt = 4 * (sb + 1)
            nc.tensor.matmul(
                z0[:, off:512],
                v_t[j][:, 2 * p, :],
                pt[:, off:512],
                start=(j == 0), stop=(j == nkt - 1),
            )
            nc.tensor.matmul(
                z1[:, off:512],
                v_t[j][:, 2 * p + 1, :],
                pt[:, 512 + off:1024],
                start=(j == 0), stop=(j == nkt - 1),
            )

        def attn_j(p, sb, j, z0, z1, fill=None):
            pt, off = attn_scores(p, sb, j)
            if fill is not None:
                fill()
            attn_pv(p, sb, j, z0, z1, pt, off)

        def attn_norm(p, sb, z0, z1, q0=0, q1=512):
            # normalize by the softmax denominator (row DH of z psum).
            # First copy z psum to SBUF so the bank frees immediately (the
            # next superblock's PV only waits for this copy, not the whole
            # reciprocal/broadcast/multiply chain). Optional [q0,q1) restricts
            # to a column chunk so the tail can pipeline norm with oproj.
            qs = (sb * SBW + q0, sb * SBW + q1)
            w = q1 - q0
            d0row = ph2.tile([1, 512], F32, tag="d0row", bufs=2, name="d0row")
            d1row = ph2.tile([1, 512], F32, tag="d1row", bufs=2, name="d1row")
            nc.vector.tensor_copy(d0row[:, 0:w], z0[DH:DH + 1, q0:q1])
            nc.vector.tensor_copy(d1row[:, 0:w], z1[DH:DH + 1, q0:q1])
            nc.vector.reciprocal_approx_fast(d0row[:, 0:w], d0row[:, 0:w])
            nc.vector.reciprocal_approx_fast(d1row[:, 0:w], d1row[:, 0:w])
            r0 = ph2.tile([64, 512], F32, tag="r0", bufs=2, name="r0")
            r1 = ph2.tile([64, 512], F32, tag="r1", bufs=2, name="r1")
            nc.gpsimd.partition_broadcast(r0[:, 0:w], d0row[:, 0:w], channels=64)
            nc.gpsimd.partition_broadcast(r1[:, 0:w], d1row[:, 0:w], channels=64)
            nc.vector.tensor_mul(z_t[p][0:64, qs[0]:qs[1]], z0[0:64, q0:q1], r0[:, 0:w])
            t1 = ph2.tile([64, 512], BF16, tag="t1", bufs=2, name="t1")
            nc.vector.tensor_mul(t1[:, 0:w], z1[0:64, q0:q1], r1[:, 0:w])
            nc.sync.dma_start(z_t[p][64:128, qs[0]:qs[1]], t1[:, 0:w])

        with (
            tc.tile_pool(name="ph1", bufs=1) as ph1,
            tc.tile_pool(name="pqk", bufs=1, space="PSUM") as pqk,
        ):
            # xt in per-512-column-block tiles: attention(0, sb) needs only
            # Q/K columns <= (sb+1)*512 (causal), so the whole front of the
            # kernel pipelines by column block.
            xt_t = [[ph1.tile([128, SBW], BF16, tag=f"xt{m}_{cb}", name=f"xt{m}_{cb}")
                     for cb in range(NSB)] for m in range(MC)]
            wv_t = [ph1.tile([128, NHC * DH], BF16, tag=f"wv{m}", name=f"wv{m}") for m in range(MC)]

            def qk_gen(p, sb_outer=False):
                """QK projection for pair p (bf16, pair-stacked partitions),
                yielded one matmul at a time for interleaving. With
                sb_outer=True the superblock loop is outermost so early
                superblocks finish as soon as their xt column block lands."""
                qt = [hold.tile([128, SBW], BF16, tag=f"qt{i}", bufs=2, name=f"qt{i}")
                      for i in range(NSB)]
                kt = [hold.tile([128, SBW], BF16, tag=f"kt{i}", bufs=2, name=f"kt{i}")
                      for i in range(NSB)]
                qts[p] = (qt, kt)
                wqk = []
                for (w_d, b_t, dst) in ((wq_d, bq_t, qt), (wk_d, bk_t, kt)):
                    wts = []
                    for m in range(MC):
                        w = ph1.tile([128, 128], BF16, tag="w", bufs=16, name="w")
                        nc.sync.dma_start(w[:], w_d.ap()[p, m])
                        wts.append(w)
                    wqk.append((wts, b_t, dst))
                order = (
                    [(sb, wb) for sb in range(NSB) for wb in wqk]
                    if sb_outer else
                    [(sb, wb) for wb in wqk for sb in range(NSB)]
                )
                for sb, (wts, b_t, dst) in order:
                    ps = pqk.tile([128, 512], F32, tag="acc", bufs=2, name="acc")
                    for m in range(MC):
                        nc.tensor.matmul(
                            ps[:],
                            wts[m][:],
                            xt_t[m][sb][:],
                            start=(m == 0),
                            stop=(m == MC - 1),
                        )
                        yield
                    nc.vector.tensor_scalar_add(dst[sb][:], ps[:], b_t[p][:])
                    yield

            # column-block pipelined prelude: per block, land xt columns,
            # then V-projection for its 4 seq tiles and pair 0's QK for it.
            g0 = qk_gen(0, sb_outer=True)
            for cb in range(NSB):
                for m in range(MC):
                    # alternate trigger queues: each dma_start costs ~0.6us of
                    # issuing-engine queue time, which otherwise serializes
                    eng = nc.scalar if m % 2 == 0 else nc.sync
                    eng.dma_start(
                        xt_t[m][cb][:],
                        xt_d.ap()[m * 128:(m + 1) * 128, cb * SBW:(cb + 1) * SBW],
                    )
                    if cb == 0:
                        eng2 = nc.sync if m % 2 == 0 else nc.scalar
                        eng2.dma_start(wv_t[m][:], wv_d.ap()[m])
                for st in range(4 * cb, 4 * cb + 4):
                    ps = pqk.tile([128, 512], F32, tag="acc", bufs=2, name="acc")
                    for m in range(MC):
                        nc.tensor.matmul(
                            ps[:],
                            xt_t[m][cb][:, (st % 4) * 128:(st % 4 + 1) * 128],
                            wv_t[m][:],
                            start=(m == 0),
                            stop=(m == MC - 1),
                        )
                    nc.vector.tensor_copy(
                        v_t[st][:, :, 0:DH],
                        ps[:].rearrange("p (h d) -> p h d", h=NHC),
                    )
                    nc.vector.tensor_copy(
                        v_t[st][:, :, DH],
                        ones_c[:].to_broadcast((128, NHC)),
                    )
                for _ in range(18):  # one QK column-block (2 proj x (8 mm + copy))
                    try:
                        next(g0)
                    except StopIteration:
                        break
            for _ in g0:
                pass

            # attention for pairs 0-2, with pair p+1's projection matmuls
            # interleaved into the ACT-paced attention stream
            for p in range(3):
                g = qk_gen(p + 1)
                state = {"done": False, "emitted": 0, "step": 0}

                def fill(state=state, g=g):
                    state["step"] += 1
                    step = state["step"]
                    want = 2 * step if step <= 32 else 64 + (step - 32)
                    while state["emitted"] < want and not state["done"]:
                        try:
                            next(g)
                            state["emitted"] += 1
                        except StopIteration:
                            state["done"] = True

                for sb in range(NSB):
                    nkt = 4 * (sb + 1)
                    z0 = patn.tile([DH + 1, 512], F32, tag="z0", bufs=1, name="z0")
                    z1 = patn.tile([DH + 1, 512], F32, tag="z1", bufs=1, name="z1")
                    for j in range(nkt):
                        attn_j(p, sb, j, z0, z1, fill=fill)
                    attn_norm(p, sb, z0, z1)
                while not state["done"]:
                    try:
                        next(g)
                    except StopIteration:
                        state["done"] = True

        # ---------------- last pair + output projection ----------------
        with (
            tc.tile_pool(name="ph3", bufs=1) as ph3,
            tc.tile_pool(name="po", bufs=1, space="PSUM") as po,
        ):
            wo_t = [ph3.tile([128, DM], BF16, tag=f"wo{p}", name=f"wo{p}") for p in range(PAIRS)]
            for p in range(PAIRS):
                nc.sync.dma_start(wo_t[p][:], wo_d.ap()[p])

            def oproj(q, mb, cp=None):
                # own psum pool (the banks freed by the closed pqk pool) so
                # units never contend with pair-3's score psum
                ps = po.tile([128, 512], F32, tag="oacc", bufs=2, name="ops")
                for p in range(PAIRS):
                    nc.tensor.matmul(
                        ps[:],
                        z_t[p][:, q * 128:(q + 1) * 128],
                        wo_t[p][:, mb * 512:(mb + 1) * 512],
                        start=(p == 0),
                        stop=(p == PAIRS - 1),
                    )
                ost = ph3.tile([128, 512], F32, tag="ost", bufs=4, name="ost")
                (cp or nc.vector.tensor_copy)(ost[:], ps[:])
                nc.sync.dma_start(
                    out_d.ap()[q * 128:(q + 1) * 128, mb * 512:(mb + 1) * 512],
                    ost[:],
                )

            # pair 3's attention, with output-projection tiles for already-
            # complete superblocks interleaved in (sb lags by one).
            otodo = [(q, mb) for q in range(NST) for mb in range(2)]
            odone = 0
            # alternate z psum between two tag sets (the second lives in the
            # banks freed by the projection pool) so superblock boundaries
            # don't stall on the previous normalize.
            for sb in range(NSB):
                nkt = 4 * (sb + 1)
                z0 = patn.tile([DH + 1, 512], F32, tag="z0", bufs=1, name="z0")
                z1 = patn.tile([DH + 1, 512], F32, tag="z1", bufs=1, name="z1")

                def fill(sb=sb):
                    # z for superblocks < sb is final for all pairs; issue up
                    # to 2 units per step, but hold 3 back so the final norm
                    # chain has PE work to hide behind
                    nonlocal odone
                    ready = min(sb * 8, 21)
                    for _ in range(2):
                        if odone < ready:
                            oproj(*otodo[odone])
                            odone += 1

                for j in range(nkt):
                    attn_j(3, sb, j, z0, z1, fill=fill)
                if sb < NSB - 1:
                    attn_norm(3, sb, z0, z1)
                else:
                    # last superblock: chunk the normalize by 128 columns and
                    # emit each q-tile's 2 output units right after its chunk,
                    # so the norm chain pipelines with the final projections
                    for c in range(4):
                        attn_norm(3, sb, z0, z1, q0=c * 128, q1=(c + 1) * 128)
                        if c == 0:
                            # drain any interleave backlog (needs only sb<3 z,
                            # which is final) while chunk 0's norm chain runs
                            while odone < 24:
                                oproj(*otodo[odone])
                                odone += 1
                        for i in range(2):
                            oproj(*otodo[odone],
                                  cp=(nc.scalar.copy if i % 2 == 0 else None))
                            odone += 1

    nc.compile()
    return nc


def _get_nc():
    if "nc" not in _NC_CACHE:
        _NC_CACHE["nc"] = _build_nc()
    return _NC_CACHE["nc"]


def _causal_masks():
    k = np.arange(128)[:, None]
    q = np.arange(128)[None, :]
    tri = np.where(q >= k, 0.0, MASK_NEG).astype(ml_dtypes.bfloat16)
    return np.concatenate([tri, tri], axis=1)  # one copy per head half


def kernel(resid_pre, W_Q, W_K, W_V, W_O, b_Q, b_K, b_V, b_O):
    global LAST_RESULTS
    resid_pre = np.asarray(resid_pre, dtype=np.float32)
    W_Q = np.asarray(W_Q, dtype=np.float32)
    W_K = np.asarray(W_K, dtype=np.float32)
    W_V = np.asarray(W_V, dtype=np.float32)
    W_O = np.asarray(W_O, dtype=np.float32)
    b_Q = np.asarray(b_Q, dtype=np.float32)
    b_K = np.asarray(b_K, dtype=np.float32)
    b_V = np.asarray(b_V, dtype=np.float32)
    b_O = np.asarray(b_O, dtype=np.float32)

    B = resid_pre.shape[0]
    masks = _causal_masks()
    ident = np.eye(128, dtype=ml_dtypes.bfloat16)

    def pack_pairs(w):  # [8, 1024, 64] -> [4, 8, 128, 128]
        return np.ascontiguousarray(
            w.reshape(PAIRS, 2, DM, DH).transpose(0, 2, 1, 3).reshape(PAIRS, MC, 128, 128)
        )

    in_maps = []
    for c in range(8):
        b, g = divmod(c, 2)
        hs = slice(g * NHC, (g + 1) * NHC)
        in_maps.append({
            "xt": np.ascontiguousarray(resid_pre[b].T).astype(ml_dtypes.bfloat16),
            "wq": pack_pairs(W_Q[hs]).astype(ml_dtypes.bfloat16),
            "wk": pack_pairs(W_K[hs]).astype(ml_dtypes.bfloat16),
            "wv": np.ascontiguousarray(
                W_V[hs].transpose(1, 0, 2).reshape(DM, NHC * DH).reshape(MC, 128, NHC * DH)
            ).astype(ml_dtypes.bfloat16),
            "wo": np.ascontiguousarray(W_O[hs].reshape(PAIRS, 128, DM)).astype(ml_dtypes.bfloat16),
            "bq": np.ascontiguousarray(b_Q[hs].reshape(PAIRS, 128, 1)),
            "bk": np.ascontiguousarray(b_K[hs].reshape(PAIRS, 128, 1)),
            "mask": masks,
            "ident": ident,
        })

    nc = _get_nc()
    res = bass_utils.run_bass_kernel_spmd(nc, in_maps, core_ids=list(range(8)))
    LAST_RESULTS = res

    # b_V contributes exactly sum_h W_O[h].T @ b_V[h] (softmax rows sum to 1)
    const = np.einsum("hdm,hd->m", W_O, b_V).astype(np.float32) + b_O
    out = np.empty((B, S, DM), dtype=np.float32)
    for b in range(B):
        out[b] = res.results[2 * b]["out"] + res.results[2 * b + 1]["out"] + const
    return out

